# revision 36
# baseline (speedup 1.0000x reference)
"""Multi-head attention (B=4, S=2048, D=1024, H=16) on 8 TRN2 NeuronCores.

Sharding: head-parallel. Core c owns heads {2c, 2c+1} (columns
[128c:128c+128] of Wq/Wk/Wv, rows of Wo) for ALL batches. Per-batch key
tile counts n_kt(b) = ceil(valid_len/128) are baked into the program, so
every core runs the same, minimal amount of attention work (sum over
batches of n_kt) instead of 8 heads x max(n_kt) as a batch-sharded
layout would. Row-parallel Wo gives per-core partial outputs [B,S,D]
summed on the host.

All matmul inputs are bf16 (inputs quantized host-side; rel err ~6e-3
vs the 2e-2 gate). Per-core dataflow:
  KT[d',s] = (Xk Wk)^T   (d' = 2*64 head dims on partitions)
  QT[d',s] = (Xq Wq)^T
  V[s,kt,h,65] = Xv Wv    (s on partitions; col 64 = ones -> denominator)
  per batch, per 512-wide q slab, per key tile kt:
    sc[k, 2, 512] = KT_h^T-slice.T @ QT_h  for both heads (two PE row groups)
    ex = Exp(sc*scale + maskbias)          (one ACT op covers both heads)
    av[0:65, h-half] += V_h.T @ ex_h       (row 64 accumulates denominator)
  normalization: denominators bounce through DRAM to land 32-wide on
  partitions for the DVE reciprocal, are broadcast via a K=1 matmul of
  ones, and multiply the head outputs into OT2[128, s] (head 1 hoisted
  to partitions 64:127 by an SBUF->SBUF DMA).
  out[s,:] partial = OT2[:, s-tile].T @ Wo2   (K=128, N=1024, bf16 out)

The PE instruction stream is kept gap-free (HAM clock at 8/8): the
projection matmuls of the next batch and the output-projection /
normalization of the previous batch are interleaved as filler between
attention rounds, so the PE never waits on the ACT engine's exp.
"""

import math

import numpy as np

B, S, D, H = 4, 2048, 1024, 16
HD = D // H  # 64
NCORES = 8
NEG = -1.0e6
P = 128

_PROG_CACHE = {}


def _patch_tile_drain():
    """The walrus build in this container rejects sem waits attached to the
    Tile end-of-kernel Drain ("Too many sync wait commands" / SIGABRT).
    Replace them with standalone EventSemaphore waits, which it accepts."""
    import concourse.tile as tile
    from concourse.vector_clock import ScopedClock

    if getattr(tile.TileContext, "_drain_patched", False):
        return

    def _drain_and_barrier(self, tick_clock, wait_clock):
        nc = self.nc
        drain_inst = nc.sync.drain()
        wait_clock.add_sem_waits(
            drain_inst.ins, ScopedClock({None: tick_clock.global_clock})
        )
        si = drain_inst.ins.sync_info
        waits = list(si.on_wait) if si is not None and si.on_wait else []
        if waits:
            si.on_wait.clear()
            by_id, by_name = {}, {}
            for h in wait_clock.sems.allocated().values():
                by_id[getattr(h, "id", None)] = h
                by_name[getattr(h, "name", None)] = h
            for w in waits:
                h = by_id.get(w.id) or by_name.get(w.ant_name)
                assert h is not None, f"no handle for sem {w.ant_name} ({w.id})"
                nc.sync.wait_ge(h, w.wait_value)
        nc.all_engine_barrier()
        assert self.sems is not None
        popped = nc._tile_sem_poison_stack.pop()
        assert popped is self._sem_poison
        nc.clear_and_free_semaphores(list(self.sems.allocated().values()))
        nc.all_engine_barrier()

    tile.TileContext._drain_and_barrier = _drain_and_barrier
    tile.TileContext._drain_patched = True


def _split_multi_waits(nc, mybir):
    """This container's walrus rejects instructions carrying more than one
    semaphore wait ("Too many sync wait commands"). Hoist excess waits into
    standalone EventSemaphore instructions on the same engine, inserted
    immediately before the instruction — same-engine stream order preserves
    the semantics exactly."""
    n_ev = 0
    for fn in nc.m.functions:
        for bb in fn.blocks:
            insts = bb.instructions
            out = []
            for inst in insts:
                si = inst.sync_info
                waits = list(si.on_wait) if si is not None and si.on_wait else []
                keep = 0 if inst.opcode == "Drain" else 1
                if len(waits) > keep:
                    excess = waits[: len(waits) - keep]
                    kept = waits[len(waits) - keep:]
                    si.on_wait.clear()
                    si.on_wait.extend(kept)
                    for w in excess:
                        ev = mybir.InstEventSemaphore(
                            name=f"{inst.name}-hw{n_ev}",
                            engine=inst.engine,
                        )
                        ev.sync_info = mybir.SyncInfo(on_wait=[w], on_update=[])
                        out.append(ev)
                        n_ev += 1
                out.append(inst)
            if n_ev:
                insts[:] = out
    return n_ev


def _build_program(n_kts: tuple):
    import concourse.bass as bass
    import concourse.mybir as mybir
    import concourse.tile as tile

    _patch_tile_drain()

    f32 = mybir.dt.float32
    f32r = mybir.dt.float32r
    bf16 = mybir.dt.bfloat16
    AF = mybir.ActivationFunctionType

    nks = sum(n_kts)
    koff = [sum(n_kts[:b]) for b in range(B)]  # packed kt-column offsets
    # smallest batch first: the biggest attention window runs last and
    # absorbs its own normalization/output-projection tail work as filler,
    # so the endgame never degenerates into a dependency-chained stall train
    order = sorted(range(B), key=lambda b: n_kts[b])

    nc = bass.Bass()

    xq_d = nc.dram_tensor("xq", [B, D, S], bf16, kind="ExternalInput")
    xk_d = nc.dram_tensor("xk", [D, nks * P], bf16, kind="ExternalInput")
    xv_d = nc.dram_tensor("xv", [D, nks * P], bf16, kind="ExternalInput")
    wq_d = nc.dram_tensor("wq", [D, P], bf16, kind="ExternalInput")
    wk_d = nc.dram_tensor("wk", [D, P], bf16, kind="ExternalInput")
    wv_d = nc.dram_tensor("wv", [D, P], bf16, kind="ExternalInput")
    wo_d = nc.dram_tensor("wo", [P, D], bf16, kind="ExternalInput")
    mb_d = nc.dram_tensor("mb", [P, nks], f32, kind="ExternalInput")
    ms_d = nc.dram_tensor("ms", [P, nks], f32, kind="ExternalInput")
    out_d = nc.dram_tensor("out", [B, 16, P, D], bf16, kind="ExternalOutput")

    with tile.TileContext(nc) as tc:
        with (
            tc.tile_pool(name="pp", bufs=1) as pp,
            tc.tile_pool(name="xsp", bufs=3) as xsp,
            tc.tile_pool(name="expp", bufs=3) as expp,
            tc.tile_pool(name="obp", bufs=3) as obp,
            tc.tile_pool(name="dnp", bufs=2) as dnp,
            tc.tile_pool(name="dnrp", bufs=3) as dnrp,
            tc.tile_pool(name="othp", bufs=2) as othp,
            tc.tile_pool(name="psMM", bufs=2, space="PSUM") as psMM,
            tc.tile_pool(name="psAV", bufs=2, space="PSUM") as psAV,
        ):
            # ---- persistent SBUF
            wq = pp.tile([P, 8, P], bf16, name="wq")
            wk = pp.tile([P, 8, P], bf16, name="wk")
            wv = pp.tile([P, 8, P], bf16, name="wv")
            wo = pp.tile([P, D], bf16, name="wo")
            ones = pp.tile([P, HD], bf16, name="ones")
            mb = pp.tile([P, nks], f32, name="mb")
            ms = pp.tile([P, nks], f32, name="ms")
            QT = [pp.tile([P, S], bf16, name=f"QT{b}") for b in range(B)]
            KT = [pp.tile([P, n_kts[b] * P], bf16, name=f"KT{b}") for b in range(B)]
            V = [pp.tile([P, n_kts[b], 2, HD + 1], bf16, name=f"V{b}") for b in range(B)]
            OT2 = [pp.tile([P, S], bf16, name=f"OT2{b}") for b in range(B)]
            avb_t = {}  # per-batch [65, 4, 2, 512] f32 tiles (pooled, bufs=2)
            dnr_t = {}  # per-batch [1, 2*S] f32 tiles (pooled, bufs=2)

            nc.any.memset(ones[:], 1.0)
            for b in range(B):
                nc.any.memset(V[b][:, :, :, HD:HD + 1], 1.0)
            nc.sync.dma_start(mb[:], mb_d[:, :])
            nc.sync.dma_start(ms[:], ms_d[:, :])
            nc.sync.dma_start(wq[:], wq_d[:, :].rearrange("(a p) c -> p a c", p=P))
            nc.sync.dma_start(wk[:], wk_d[:, :].rearrange("(a p) c -> p a c", p=P))
            nc.sync.dma_start(wv[:], wv_d[:, :].rearrange("(a p) c -> p a c", p=P))
            nc.sync.dma_start(wo[:], wo_d[:, :])

            xk_re = xk_d[:, :].rearrange("(a p) s -> p a s", p=P)
            xv_re = xv_d[:, :].rearrange("(a p) s -> p a s", p=P)

            ob_i = [0]  # alternates od copies between DVE and ACT

            # ---------- thunk builders (each thunk = one schedulable step) ----
            def a_thunks(b):
                """Projection phase for batch b: list of (dma_thunk, mm_thunk)
                pairs flattened with 2-ahead DMA prefetch."""
                dmas, mms = [], []
                xq_re = xq_d[b, :, :].rearrange("(a p) s -> p a s", p=P)
                nb = n_kts[b] * P

                # KT: (Xk Wk)^T in 512-col chunks
                for off in range(0, nb, 512):
                    cw = min(512, nb - off)
                    xs = [None]

                    def dk(off=off, cw=cw, xs=xs):
                        xs[0] = xsp.tile([P, 8, 512], bf16, name="xs", tag="xs")
                        nc.sync.dma_start(
                            xs[0][:, :, 0:cw],
                            xk_re[:, :, koff[b] * P + off:koff[b] * P + off + cw],
                        )

                    def mk(b=b, off=off, cw=cw, xs=xs):
                        pk = psMM.tile([P, 1024], f32, name="pk", tag="mm")
                        for a in range(8):
                            nc.tensor.matmul(
                                pk[:, 0:cw],
                                lhsT=wk[:, a, :],
                                rhs=xs[0][:, a, 0:cw],
                                start=(a == 0),
                                stop=(a == 7),
                            )
                        nc.vector.tensor_copy(
                            out=KT[b][:, off:off + cw], in_=pk[:, 0:cw]
                        )

                    dmas.append(dk)
                    mms.append(mk)

                # V: Xv Wv natural layout, 512-col chunks = 4 key tiles each
                for off in range(0, nb, 512):
                    cw = min(512, nb - off)
                    nst = cw // P
                    xs = [None]

                    def dv(off=off, cw=cw, xs=xs):
                        xs[0] = xsp.tile([P, 8, 512], bf16, name="xs", tag="xs")
                        nc.sync.dma_start(
                            xs[0][:, :, 0:cw],
                            xv_re[:, :, koff[b] * P + off:koff[b] * P + off + cw],
                        )

                    def mv(b=b, off=off, nst=nst, xs=xs):
                        pv = psMM.tile([P, 1024], f32, name="pv", tag="mm")
                        for st in range(nst):
                            for a in range(8):
                                nc.tensor.matmul(
                                    pv[:, st * P:(st + 1) * P],
                                    lhsT=xs[0][:, a, st * P:(st + 1) * P],
                                    rhs=wv[:, a, :],
                                    start=(a == 0),
                                    stop=(a == 7),
                                )
                        kt0 = off // P
                        nc.vector.tensor_copy(
                            out=V[b][:, kt0:kt0 + nst, :, 0:HD],
                            in_=pv[:, 0:nst * P].rearrange(
                                "p (st h c) -> p st h c", st=nst, h=2
                            ),
                        )

                    dmas.append(dv)
                    mms.append(mv)

                # QT: (Xq Wq)^T in 512-col chunks (4 chunks)
                for off in range(0, S, 512):
                    xs = [None]

                    def dq(b=b, off=off, xs=xs):
                        xs[0] = xsp.tile([P, 8, 512], bf16, name="xs", tag="xs")
                        nc.sync.dma_start(xs[0][:], xq_re[:, :, off:off + 512])

                    def mq(b=b, off=off, xs=xs):
                        pq = psMM.tile([P, 1024], f32, name="pq", tag="mm")
                        for a in range(8):
                            nc.tensor.matmul(
                                pq[:, 0:512],
                                lhsT=wq[:, a, :],
                                rhs=xs[0][:, a, :],
                                start=(a == 0),
                                stop=(a == 7),
                            )
                        nc.vector.tensor_copy(
                            out=QT[b][:, off:off + 512], in_=pq[:, 0:512]
                        )

                    dmas.append(dq)
                    mms.append(mq)

                # interleave: keep 2 DMAs in flight ahead of their consumers
                thunks = []
                nd = len(dmas)
                thunks.append(dmas[0])
                if nd > 1:
                    thunks.append(dmas[1])
                for i in range(nd):
                    thunks.append(mms[i])
                    if i + 2 < nd:
                        thunks.append(dmas[i + 2])
                return thunks

            oth_t = {}  # per-batch [HD, S] bf16 head-1 staging (pooled)

            def slab_tail_thunks(b, slab):
                """Normalize + output-project one finished 512-q slab of batch
                b. Returns PE-bearing thunks for the filler queue; the
                denominator reciprocal chain (DMA/DVE only) is emitted inline
                by the caller so its latency hides behind later rounds."""
                q0 = slab * 512
                ts = []

                def tm(b=b, slab=slab, q0=q0):
                    bc = psMM.tile([P, 1024], f32, name="bc", tag="mm")
                    if slab == 0:
                        oth_t[b] = othp.tile([HD, S], bf16, name="oth", tag="oth")
                    for h in range(2):
                        nc.tensor.matmul(
                            bc[0:HD, h * 512:(h + 1) * 512],
                            lhsT=ones[0:1, :],
                            rhs=dnr_t[(b, slab)][0:1, h, :],
                            start=True,
                            stop=True,
                        )
                    for h in range(2):
                        dst = OT2[b] if h == 0 else oth_t[b]
                        nc.vector.tensor_mul(
                            out=dst[0:HD, q0:q0 + 512],
                            in0=avb_t[b][0:HD, slab, h, :],
                            in1=bc[0:HD, h * 512:(h + 1) * 512],
                        )
                    nc.sync.dma_start(
                        OT2[b][HD:P, q0:q0 + 512], oth_t[b][:, q0:q0 + 512]
                    )

                ts.append(tm)
                for st in range(4 * slab, 4 * slab + 4):

                    def td(b=b, st=st):
                        od = psMM.tile([P, 1024], f32, name="od", tag="mm")
                        for u in range(2):  # ISA moving-operand max is 512
                            nc.tensor.matmul(
                                od[:, u * 512:(u + 1) * 512],
                                lhsT=OT2[b][:, st * P:(st + 1) * P],
                                rhs=wo[:, u * 512:(u + 1) * 512],
                                start=True,
                                stop=True,
                            )
                        ob = obp.tile([P, 1024], bf16, name="ob", tag="ob")
                        if ob_i[0] % 2 == 0:
                            nc.vector.tensor_copy(out=ob[:], in_=od[:])
                        else:
                            nc.scalar.copy(ob[:], od[:])
                        ob_i[0] += 1
                        nc.sync.dma_start(out_d[b, st, :, :], ob[:])

                    ts.append(td)
                return ts

            # ---------- attention phase with filler interleaving ------------
            gr = [0]  # global round counter (paces filler readiness)

            def run_c(b, fill):
                """Attention for batch b; pops filler thunks between rounds so
                the PE stream never drains while ACT computes exp. Entries are
                (ready_round, thunk): a thunk only pops once the global round
                counter passes ready_round, keeping chain latencies hidden."""
                nkt = n_kts[b]
                rounds = 4 * nkt
                r = 0
                for slab in range(4):
                    q0 = slab * 512
                    avs = psAV.tile([HD + 1, 1024], f32, name="avs", tag="av")
                    pend = None  # (kt, ex) awaiting its AV matmuls
                    for kt in range(nkt):
                        sc = psMM.tile([P, 1024], f32, name="sc", tag="mm")
                        for h in range(2):
                            nc.tensor.matmul(
                                sc[:, h * 512:(h + 1) * 512],
                                lhsT=KT[b][h * HD:(h + 1) * HD, kt * P:(kt + 1) * P],
                                rhs=QT[b][h * HD:(h + 1) * HD, q0:q0 + 512],
                                start=True,
                                stop=True,
                            )
                        ex = expp.tile([P, 1024], bf16, name="ex", tag="ex")
                        col = koff[b] + kt
                        nc.scalar.activation(
                            ex[:],
                            sc[:],
                            AF.Exp,
                            bias=mb[:, col:col + 1],
                            scale=ms[:, col:col + 1],
                        )
                        if pend is not None:
                            pkt, pex = pend
                            for h in range(2):
                                nc.tensor.matmul(
                                    avs[:, h * 512:(h + 1) * 512],
                                    lhsT=V[b][:, pkt, h, :],
                                    rhs=pex[:, h * 512:(h + 1) * 512],
                                    start=(pkt == 0),
                                    stop=False,
                                )
                        pend = (kt, ex)
                        # filler keeps the PE busy while ACT runs exp; pop the
                        # first READY entry (skip entries still inside their
                        # producer chain's latency window — same-group entries
                        # share a ready round, so group order is preserved)
                        rem_rounds = rounds - r
                        quota = (len(fill) + rem_rounds - 1) // rem_rounds
                        done = 0
                        while fill and done < quota:
                            idx = next(
                                (k for k, e in enumerate(fill) if e[0] <= gr[0]),
                                None,
                            )
                            if idx is None:
                                break
                            fill.pop(idx)[1]()
                            done += 1
                        r += 1
                        gr[0] += 1
                    pkt, pex = pend
                    for h in range(2):
                        nc.tensor.matmul(
                            avs[:, h * 512:(h + 1) * 512],
                            lhsT=V[b][:, pkt, h, :],
                            rhs=pex[:, h * 512:(h + 1) * 512],
                            start=(pkt == 0),
                            stop=True,
                        )
                    # release the PSUM tile: head outputs and denominator row
                    if slab == 0:
                        avb_t[b] = dnp.tile(
                            [HD + 1, 4, 2, 512], f32, name="avb", tag="avb"
                        )
                    nc.vector.tensor_copy(
                        out=avb_t[b][:, slab, :, :],
                        in_=avs[:, :].rearrange("p (h q) -> p h q", h=2),
                    )
                    # denominator reciprocal: bounce the row across 8
                    # partitions (SBUF->SBUF DMA) so the 6-cycle/elem DVE
                    # reciprocal runs 8-wide, then collapse back. The
                    # dependent broadcast matmul is delayed ~4 rounds in the
                    # filler queue so this chain's latency stays hidden.
                    dnT = dnrp.tile([8, P], f32, name="dnT", tag="dnT")
                    dnTr = dnrp.tile([8, P], bf16, name="dnTr", tag="dnTr")
                    nc.sync.dma_start(dnT[:], avb_t[b][HD:HD + 1, slab, :, :])
                    with nc.allow_low_precision(
                        reason="softmax denominators are O(1e3); bf16 "
                        "reciprocal keeps enough digits for attention"
                    ):
                        nc.vector.reciprocal(dnTr[:], dnT[:])
                    dnr = dnrp.tile([1, 2, 512], bf16, name="dnr", tag="dnr")
                    nc.sync.dma_start(dnr[:], dnTr[:])
                    dnr_t[(b, slab)] = dnr
                    fill.extend(
                        (gr[0] + 4, t) for t in slab_tail_thunks(b, slab)
                    )

            # ---------- schedule ---------------------------------------------
            fill = []
            for t in a_thunks(order[0]):
                t()
            for i, b in enumerate(order):
                if i + 1 < B:
                    fill.extend((0, t) for t in a_thunks(order[i + 1]))
                run_c(b, fill)
            while fill:
                fill.pop(0)[1]()

    _split_multi_waits(nc, mybir)
    return nc


def _zip(l1, l2):
    out = []
    for i in range(max(len(l1), len(l2))):
        if i < len(l1):
            out.append(l1[i])
        if i < len(l2):
            out.append(l2[i])
    return out


def _get_program(n_kts: tuple):
    if n_kts not in _PROG_CACHE:
        _PROG_CACHE[n_kts] = _build_program(n_kts)
    return _PROG_CACHE[n_kts]


def kernel(**inputs) -> np.ndarray:
    import ml_dtypes
    from concourse.bass_utils import run_bass_kernel_spmd

    bf = ml_dtypes.bfloat16
    q = np.asarray(inputs["queries"], dtype=np.float32)
    k = np.asarray(inputs["keys"], dtype=np.float32)
    v = np.asarray(inputs["values"], dtype=np.float32)
    vl = np.asarray(inputs["valid_lens"]).astype(np.int64)
    Wq = np.asarray(inputs["Wq"], dtype=np.float32)
    Wk = np.asarray(inputs["Wk"], dtype=np.float32)
    Wv = np.asarray(inputs["Wv"], dtype=np.float32)
    Wo = np.asarray(inputs["Wo"], dtype=np.float32)

    n_kts = tuple(
        (S // P) if int(vl[b]) == 0 else min(S // P, math.ceil(int(vl[b]) / P))
        for b in range(B)
    )
    nc = _get_program(n_kts)
    nks = sum(n_kts)

    # shared (core-independent) input tensors
    xq = np.ascontiguousarray(q.transpose(0, 2, 1)).astype(bf)
    xk = np.concatenate(
        [k[b].T[:, :n_kts[b] * P] for b in range(B)], axis=1
    ).astype(bf)
    xv = np.concatenate(
        [v[b].T[:, :n_kts[b] * P] for b in range(B)], axis=1
    ).astype(bf)
    mbs, mss = [], []
    for b in range(B):
        kk = (np.arange(n_kts[b])[None, :] * P + np.arange(P)[:, None]).astype(
            np.int64
        )
        vlb = int(vl[b])
        if vlb == 0:
            mbs.append(np.zeros((P, n_kts[b]), np.float32))
            mss.append(np.zeros((P, n_kts[b]), np.float32))
        else:
            mbs.append(np.where(kk < vlb, 0.0, NEG).astype(np.float32))
            mss.append(np.full((P, n_kts[b]), 1.0 / math.sqrt(HD), np.float32))
    mb = np.concatenate(mbs, axis=1)
    ms = np.concatenate(mss, axis=1)

    in_maps = []
    for c in range(NCORES):
        cols = slice(c * P, (c + 1) * P)
        in_maps.append(
            {
                "xq": xq,
                "xk": xk,
                "xv": xv,
                "wq": np.ascontiguousarray(Wq[:, cols]).astype(bf),
                "wk": np.ascontiguousarray(Wk[:, cols]).astype(bf),
                "wv": np.ascontiguousarray(Wv[:, cols]).astype(bf),
                "wo": np.ascontiguousarray(Wo[cols, :]).astype(bf),
                "mb": mb,
                "ms": ms,
            }
        )

    globals()["_LAST_IN_MAPS"] = in_maps
    res = run_bass_kernel_spmd(nc, in_maps, list(range(NCORES))).results

    out = np.zeros((B, S, D), dtype=np.float32)
    for c in range(NCORES):
        out += res[c]["out"].reshape(B, S, D).astype(np.float32)
    return out


# revision 40
# speedup vs baseline: 1.0155x; 1.0155x over previous
"""Multi-head attention (B=4, S=2048, D=1024, H=16) on 8 TRN2 NeuronCores.

Sharding: head-parallel. Core c owns heads {2c, 2c+1} (columns
[128c:128c+128] of Wq/Wk/Wv, rows of Wo) for ALL batches. Per-batch key
tile counts n_kt(b) = ceil(valid_len/128) are baked into the program, so
every core runs the same, minimal amount of attention work (sum over
batches of n_kt) instead of 8 heads x max(n_kt) as a batch-sharded
layout would. Row-parallel Wo gives per-core partial outputs [B,S,D]
summed on the host.

All matmul inputs are bf16 (inputs quantized host-side; rel err ~6e-3
vs the 2e-2 gate). Per-core dataflow:
  KT[d',s] = (Xk Wk)^T   (d' = 2*64 head dims on partitions)
  QT[d',s] = (Xq Wq)^T
  V[s,kt,h,65] = Xv Wv    (s on partitions; col 64 = ones -> denominator)
  per batch, per 512-wide q slab, per key tile kt:
    sc[k, 2, 512] = KT_h^T-slice.T @ QT_h  for both heads (two PE row groups)
    ex = Exp(sc*scale + maskbias)          (one ACT op covers both heads)
    av[0:65, h-half] += V_h.T @ ex_h       (row 64 accumulates denominator)
  normalization: denominators bounce through DRAM to land 32-wide on
  partitions for the DVE reciprocal, are broadcast via a K=1 matmul of
  ones, and multiply the head outputs into OT2[128, s] (head 1 hoisted
  to partitions 64:127 by an SBUF->SBUF DMA).
  out[s,:] partial = OT2[:, s-tile].T @ Wo2   (K=128, N=1024, bf16 out)

The PE instruction stream is kept gap-free (HAM clock at 8/8): the
projection matmuls of the next batch and the output-projection /
normalization of the previous batch are interleaved as filler between
attention rounds, so the PE never waits on the ACT engine's exp.
"""

import math

import numpy as np

B, S, D, H = 4, 2048, 1024, 16
HD = D // H  # 64
NCORES = 8
NEG = -1.0e6
P = 128

_PROG_CACHE = {}


def _patch_tile_drain():
    """The walrus build in this container rejects sem waits attached to the
    Tile end-of-kernel Drain ("Too many sync wait commands" / SIGABRT).
    Replace them with standalone EventSemaphore waits, which it accepts."""
    import concourse.tile as tile
    from concourse.vector_clock import ScopedClock

    if getattr(tile.TileContext, "_drain_patched", False):
        return

    def _drain_and_barrier(self, tick_clock, wait_clock):
        nc = self.nc
        drain_inst = nc.sync.drain()
        wait_clock.add_sem_waits(
            drain_inst.ins, ScopedClock({None: tick_clock.global_clock})
        )
        si = drain_inst.ins.sync_info
        waits = list(si.on_wait) if si is not None and si.on_wait else []
        if waits:
            si.on_wait.clear()
            by_id, by_name = {}, {}
            for h in wait_clock.sems.allocated().values():
                by_id[getattr(h, "id", None)] = h
                by_name[getattr(h, "name", None)] = h
            for w in waits:
                h = by_id.get(w.id) or by_name.get(w.ant_name)
                assert h is not None, f"no handle for sem {w.ant_name} ({w.id})"
                nc.sync.wait_ge(h, w.wait_value)
        nc.all_engine_barrier()
        assert self.sems is not None
        popped = nc._tile_sem_poison_stack.pop()
        assert popped is self._sem_poison
        nc.clear_and_free_semaphores(list(self.sems.allocated().values()))
        nc.all_engine_barrier()

    tile.TileContext._drain_and_barrier = _drain_and_barrier
    tile.TileContext._drain_patched = True


def _split_multi_waits(nc, mybir):
    """This container's walrus rejects instructions carrying more than one
    semaphore wait ("Too many sync wait commands"). Hoist excess waits into
    standalone EventSemaphore instructions on the same engine, inserted
    immediately before the instruction — same-engine stream order preserves
    the semantics exactly."""
    n_ev = 0
    for fn in nc.m.functions:
        for bb in fn.blocks:
            insts = bb.instructions
            out = []
            for inst in insts:
                si = inst.sync_info
                waits = list(si.on_wait) if si is not None and si.on_wait else []
                keep = 0 if inst.opcode == "Drain" else 1
                if len(waits) > keep:
                    excess = waits[: len(waits) - keep]
                    kept = waits[len(waits) - keep:]
                    si.on_wait.clear()
                    si.on_wait.extend(kept)
                    for w in excess:
                        ev = mybir.InstEventSemaphore(
                            name=f"{inst.name}-hw{n_ev}",
                            engine=inst.engine,
                        )
                        ev.sync_info = mybir.SyncInfo(on_wait=[w], on_update=[])
                        out.append(ev)
                        n_ev += 1
                out.append(inst)
            if n_ev:
                insts[:] = out
    return n_ev


def _build_program(n_kts: tuple):
    import concourse.bass as bass
    import concourse.mybir as mybir
    import concourse.tile as tile

    _patch_tile_drain()

    f32 = mybir.dt.float32
    f32r = mybir.dt.float32r
    bf16 = mybir.dt.bfloat16
    AF = mybir.ActivationFunctionType

    nks = sum(n_kts)
    koff = [sum(n_kts[:b]) for b in range(B)]  # packed kt-column offsets
    # biggest batch first: its wide attention window hides the next batch's
    # projection DMAs, and the accumulated tail work drains through the
    # remaining windows
    order = sorted(range(B), key=lambda b: -n_kts[b])

    nc = bass.Bass()

    xq_d = nc.dram_tensor("xq", [B, D, S], bf16, kind="ExternalInput")
    xk_d = nc.dram_tensor("xk", [D, nks * P], bf16, kind="ExternalInput")
    xv_d = nc.dram_tensor("xv", [D, nks * P], bf16, kind="ExternalInput")
    wq_d = nc.dram_tensor("wq", [D, P], bf16, kind="ExternalInput")
    wk_d = nc.dram_tensor("wk", [D, P], bf16, kind="ExternalInput")
    wv_d = nc.dram_tensor("wv", [D, P], bf16, kind="ExternalInput")
    wo_d = nc.dram_tensor("wo", [P, D], bf16, kind="ExternalInput")
    mb_d = nc.dram_tensor("mb", [P, nks], f32, kind="ExternalInput")
    ms_d = nc.dram_tensor("ms", [P, nks], f32, kind="ExternalInput")
    out_d = nc.dram_tensor("out", [B, 16, P, D], bf16, kind="ExternalOutput")

    with tile.TileContext(nc) as tc:
        with (
            tc.tile_pool(name="pp", bufs=1) as pp,
            tc.tile_pool(name="xsp", bufs=6) as xsp,
            tc.tile_pool(name="expp", bufs=3) as expp,
            tc.tile_pool(name="obp", bufs=3) as obp,
            tc.tile_pool(name="dnp", bufs=2) as dnp,
            tc.tile_pool(name="dnrp", bufs=3) as dnrp,
            tc.tile_pool(name="othp", bufs=2) as othp,
            tc.tile_pool(name="psMM", bufs=2, space="PSUM") as psMM,
            tc.tile_pool(name="psAV", bufs=2, space="PSUM") as psAV,
        ):
            # ---- persistent SBUF
            wq = pp.tile([P, 8, P], bf16, name="wq")
            wk = pp.tile([P, 8, P], bf16, name="wk")
            wv = pp.tile([P, 8, P], bf16, name="wv")
            wo = pp.tile([P, D], bf16, name="wo")
            ones = pp.tile([P, HD], bf16, name="ones")
            mb = pp.tile([P, nks], f32, name="mb")
            ms = pp.tile([P, nks], f32, name="ms")
            QT = [pp.tile([P, S], bf16, name=f"QT{b}") for b in range(B)]
            KT = [pp.tile([P, n_kts[b] * P], bf16, name=f"KT{b}") for b in range(B)]
            V = [pp.tile([P, n_kts[b], 2, HD + 1], bf16, name=f"V{b}") for b in range(B)]
            OT2 = [pp.tile([P, S], bf16, name=f"OT2{b}") for b in range(B)]
            avb_t = {}  # per-batch [65, 4, 2, 512] f32 tiles (pooled, bufs=2)
            dnr_t = {}  # per-batch [1, 2*S] f32 tiles (pooled, bufs=2)

            nc.any.memset(ones[:], 1.0)
            for b in range(B):
                nc.any.memset(V[b][:, :, :, HD:HD + 1], 1.0)
            nc.sync.dma_start(mb[:], mb_d[:, :])
            nc.sync.dma_start(ms[:], ms_d[:, :])
            nc.sync.dma_start(wq[:], wq_d[:, :].rearrange("(a p) c -> p a c", p=P))
            nc.sync.dma_start(wk[:], wk_d[:, :].rearrange("(a p) c -> p a c", p=P))
            nc.sync.dma_start(wv[:], wv_d[:, :].rearrange("(a p) c -> p a c", p=P))
            nc.sync.dma_start(wo[:], wo_d[:, :])

            xk_re = xk_d[:, :].rearrange("(a p) s -> p a s", p=P)
            xv_re = xv_d[:, :].rearrange("(a p) s -> p a s", p=P)

            ob_i = [0]  # alternates od copies between DVE and ACT

            # ---------- thunk builders (each thunk = one schedulable step) ----
            def a_thunks(b):
                """Projection phase for batch b: list of (dma_thunk, mm_thunk)
                pairs flattened with 2-ahead DMA prefetch."""
                dmas, mms = [], []
                xq_re = xq_d[b, :, :].rearrange("(a p) s -> p a s", p=P)
                nb = n_kts[b] * P

                # KT: (Xk Wk)^T in 512-col chunks
                for off in range(0, nb, 512):
                    cw = min(512, nb - off)
                    xs = [None]

                    def dk(off=off, cw=cw, xs=xs):
                        xs[0] = xsp.tile([P, 8, 512], bf16, name="xs", tag="xs")
                        nc.sync.dma_start(
                            xs[0][:, :, 0:cw],
                            xk_re[:, :, koff[b] * P + off:koff[b] * P + off + cw],
                        )

                    def mk(b=b, off=off, cw=cw, xs=xs):
                        pk = psMM.tile([P, 1024], f32, name="pk", tag="mm")
                        for a in range(8):
                            nc.tensor.matmul(
                                pk[:, 0:cw],
                                lhsT=wk[:, a, :],
                                rhs=xs[0][:, a, 0:cw],
                                start=(a == 0),
                                stop=(a == 7),
                            )
                        nc.vector.tensor_copy(
                            out=KT[b][:, off:off + cw], in_=pk[:, 0:cw]
                        )

                    dmas.append(dk)
                    mms.append(mk)

                # V: Xv Wv natural layout, 512-col chunks = 4 key tiles each
                for off in range(0, nb, 512):
                    cw = min(512, nb - off)
                    nst = cw // P
                    xs = [None]

                    def dv(off=off, cw=cw, xs=xs):
                        xs[0] = xsp.tile([P, 8, 512], bf16, name="xs", tag="xs")
                        nc.sync.dma_start(
                            xs[0][:, :, 0:cw],
                            xv_re[:, :, koff[b] * P + off:koff[b] * P + off + cw],
                        )

                    def mv(b=b, off=off, nst=nst, xs=xs):
                        pv = psMM.tile([P, 1024], f32, name="pv", tag="mm")
                        for st in range(nst):
                            for a in range(8):
                                nc.tensor.matmul(
                                    pv[:, st * P:(st + 1) * P],
                                    lhsT=xs[0][:, a, st * P:(st + 1) * P],
                                    rhs=wv[:, a, :],
                                    start=(a == 0),
                                    stop=(a == 7),
                                )
                        kt0 = off // P
                        nc.vector.tensor_copy(
                            out=V[b][:, kt0:kt0 + nst, :, 0:HD],
                            in_=pv[:, 0:nst * P].rearrange(
                                "p (st h c) -> p st h c", st=nst, h=2
                            ),
                        )

                    dmas.append(dv)
                    mms.append(mv)

                # QT: (Xq Wq)^T in 512-col chunks (4 chunks)
                for off in range(0, S, 512):
                    xs = [None]

                    def dq(b=b, off=off, xs=xs):
                        xs[0] = xsp.tile([P, 8, 512], bf16, name="xs", tag="xs")
                        nc.sync.dma_start(xs[0][:], xq_re[:, :, off:off + 512])

                    def mq(b=b, off=off, xs=xs):
                        pq = psMM.tile([P, 1024], f32, name="pq", tag="mm")
                        for a in range(8):
                            nc.tensor.matmul(
                                pq[:, 0:512],
                                lhsT=wq[:, a, :],
                                rhs=xs[0][:, a, :],
                                start=(a == 0),
                                stop=(a == 7),
                            )
                        nc.vector.tensor_copy(
                            out=QT[b][:, off:off + 512], in_=pq[:, 0:512]
                        )

                    dmas.append(dq)
                    mms.append(mq)

                # interleave: keep 4 DMAs in flight ahead of their consumers
                ahead = 4
                thunks = []
                nd = len(dmas)
                for i in range(min(ahead, nd)):
                    thunks.append(dmas[i])
                for i in range(nd):
                    thunks.append(mms[i])
                    if i + ahead < nd:
                        thunks.append(dmas[i + ahead])
                return thunks

            oth_t = {}  # per-batch [HD, S] bf16 head-1 staging (pooled)

            def slab_tail_thunks(b, slab):
                """Normalize + output-project one finished 512-q slab of batch
                b. Returns PE-bearing thunks for the filler queue; the
                denominator reciprocal chain (DMA/DVE only) is emitted inline
                by the caller so its latency hides behind later rounds."""
                q0 = slab * 512
                ts = []

                def tm(b=b, slab=slab, q0=q0):
                    bc = psMM.tile([P, 1024], f32, name="bc", tag="mm")
                    if slab == 0:
                        oth_t[b] = othp.tile([HD, S], bf16, name="oth", tag="oth")
                    for h in range(2):
                        nc.tensor.matmul(
                            bc[0:HD, h * 512:(h + 1) * 512],
                            lhsT=ones[0:1, :],
                            rhs=dnr_t[(b, slab)][0:1, h, :],
                            start=True,
                            stop=True,
                        )
                    for h in range(2):
                        dst = OT2[b] if h == 0 else oth_t[b]
                        nc.vector.tensor_mul(
                            out=dst[0:HD, q0:q0 + 512],
                            in0=avb_t[b][0:HD, slab, h, :],
                            in1=bc[0:HD, h * 512:(h + 1) * 512],
                        )
                    nc.sync.dma_start(
                        OT2[b][HD:P, q0:q0 + 512], oth_t[b][:, q0:q0 + 512]
                    )

                ts.append(tm)
                for st in range(4 * slab, 4 * slab + 4):

                    def td(b=b, st=st):
                        od = psMM.tile([P, 1024], f32, name="od", tag="mm")
                        for u in range(2):  # ISA moving-operand max is 512
                            nc.tensor.matmul(
                                od[:, u * 512:(u + 1) * 512],
                                lhsT=OT2[b][:, st * P:(st + 1) * P],
                                rhs=wo[:, u * 512:(u + 1) * 512],
                                start=True,
                                stop=True,
                            )
                        ob = obp.tile([P, 1024], bf16, name="ob", tag="ob")
                        # half on DVE, half on ACT: the copy is what gates the
                        # PSUM ring, so run both engines on one tile in parallel
                        nc.vector.tensor_copy(out=ob[:, 0:512], in_=od[:, 0:512])
                        nc.scalar.copy(ob[:, 512:1024], od[:, 512:1024])
                        nc.sync.dma_start(out_d[b, st, :, :], ob[:])

                    ts.append(td)
                return ts

            # ---------- attention phase with filler interleaving ------------
            gr = [0]  # global round counter (paces filler readiness)

            def run_c(b, fill):
                """Attention for batch b; pops filler thunks between rounds so
                the PE stream never drains while ACT computes exp. Entries are
                (ready_round, thunk): a thunk only pops once the global round
                counter passes ready_round, keeping chain latencies hidden."""
                nkt = n_kts[b]
                rounds = 4 * nkt
                r = 0
                for slab in range(4):
                    q0 = slab * 512
                    avs = psAV.tile([HD + 1, 1024], f32, name="avs", tag="av")
                    pend = None  # (kt, ex) awaiting its AV matmuls
                    for kt in range(nkt):
                        sc = psMM.tile([P, 1024], f32, name="sc", tag="mm")
                        for h in range(2):
                            nc.tensor.matmul(
                                sc[:, h * 512:(h + 1) * 512],
                                lhsT=KT[b][h * HD:(h + 1) * HD, kt * P:(kt + 1) * P],
                                rhs=QT[b][h * HD:(h + 1) * HD, q0:q0 + 512],
                                start=True,
                                stop=True,
                            )
                        ex = expp.tile([P, 1024], bf16, name="ex", tag="ex")
                        col = koff[b] + kt
                        nc.scalar.activation(
                            ex[:],
                            sc[:],
                            AF.Exp,
                            bias=mb[:, col:col + 1],
                            scale=ms[:, col:col + 1],
                        )
                        if pend is not None:
                            pkt, pex = pend
                            for h in range(2):
                                nc.tensor.matmul(
                                    avs[:, h * 512:(h + 1) * 512],
                                    lhsT=V[b][:, pkt, h, :],
                                    rhs=pex[:, h * 512:(h + 1) * 512],
                                    start=(pkt == 0),
                                    stop=False,
                                )
                        pend = (kt, ex)
                        # filler keeps the PE busy while ACT runs exp; pop the
                        # first READY entry (skip entries still inside their
                        # producer chain's latency window — same-group entries
                        # share a ready round, so group order is preserved)
                        rem_rounds = rounds - r
                        quota = (len(fill) + rem_rounds - 1) // rem_rounds
                        done = 0
                        while fill and done < quota:
                            idx = next(
                                (k for k, e in enumerate(fill) if e[0] <= gr[0]),
                                None,
                            )
                            if idx is None:
                                break
                            fill.pop(idx)[1]()
                            done += 1
                        r += 1
                        gr[0] += 1
                    pkt, pex = pend
                    for h in range(2):
                        nc.tensor.matmul(
                            avs[:, h * 512:(h + 1) * 512],
                            lhsT=V[b][:, pkt, h, :],
                            rhs=pex[:, h * 512:(h + 1) * 512],
                            start=(pkt == 0),
                            stop=True,
                        )
                    # release the PSUM tile: head outputs and denominator row
                    if slab == 0:
                        avb_t[b] = dnp.tile(
                            [HD + 1, 4, 2, 512], f32, name="avb", tag="avb"
                        )
                    nc.vector.tensor_copy(
                        out=avb_t[b][:, slab, :, :],
                        in_=avs[:, :].rearrange("p (h q) -> p h q", h=2),
                    )
                    # denominator reciprocal: bounce the row across 8
                    # partitions (SBUF->SBUF DMA) so the 6-cycle/elem DVE
                    # reciprocal runs 8-wide, then collapse back. The
                    # dependent broadcast matmul is delayed ~4 rounds in the
                    # filler queue so this chain's latency stays hidden.
                    dnT = dnrp.tile([8, P], f32, name="dnT", tag="dnT")
                    dnTr = dnrp.tile([8, P], bf16, name="dnTr", tag="dnTr")
                    nc.sync.dma_start(dnT[:], avb_t[b][HD:HD + 1, slab, :, :])
                    with nc.allow_low_precision(
                        reason="softmax denominators are O(1e3); bf16 "
                        "reciprocal keeps enough digits for attention"
                    ):
                        nc.vector.reciprocal(dnTr[:], dnT[:])
                    dnr = dnrp.tile([1, 2, 512], bf16, name="dnr", tag="dnr")
                    nc.sync.dma_start(dnr[:], dnTr[:])
                    dnr_t[(b, slab)] = dnr
                    fill.extend(
                        (gr[0] + 4, t) for t in slab_tail_thunks(b, slab)
                    )

            # ---------- schedule ---------------------------------------------
            fill = []
            for t in a_thunks(order[0]):
                t()
            for i, b in enumerate(order):
                if i + 1 < B:
                    fill.extend((0, t) for t in a_thunks(order[i + 1]))
                run_c(b, fill)
            while fill:
                fill.pop(0)[1]()

    _split_multi_waits(nc, mybir)
    return nc


def _zip(l1, l2):
    out = []
    for i in range(max(len(l1), len(l2))):
        if i < len(l1):
            out.append(l1[i])
        if i < len(l2):
            out.append(l2[i])
    return out


def _get_program(n_kts: tuple):
    if n_kts not in _PROG_CACHE:
        _PROG_CACHE[n_kts] = _build_program(n_kts)
    return _PROG_CACHE[n_kts]


def kernel(**inputs) -> np.ndarray:
    import ml_dtypes
    from concourse.bass_utils import run_bass_kernel_spmd

    bf = ml_dtypes.bfloat16
    q = np.asarray(inputs["queries"], dtype=np.float32)
    k = np.asarray(inputs["keys"], dtype=np.float32)
    v = np.asarray(inputs["values"], dtype=np.float32)
    vl = np.asarray(inputs["valid_lens"]).astype(np.int64)
    Wq = np.asarray(inputs["Wq"], dtype=np.float32)
    Wk = np.asarray(inputs["Wk"], dtype=np.float32)
    Wv = np.asarray(inputs["Wv"], dtype=np.float32)
    Wo = np.asarray(inputs["Wo"], dtype=np.float32)

    n_kts = tuple(
        (S // P) if int(vl[b]) == 0 else min(S // P, math.ceil(int(vl[b]) / P))
        for b in range(B)
    )
    nc = _get_program(n_kts)
    nks = sum(n_kts)

    # shared (core-independent) input tensors
    xq = np.ascontiguousarray(q.transpose(0, 2, 1)).astype(bf)
    xk = np.concatenate(
        [k[b].T[:, :n_kts[b] * P] for b in range(B)], axis=1
    ).astype(bf)
    xv = np.concatenate(
        [v[b].T[:, :n_kts[b] * P] for b in range(B)], axis=1
    ).astype(bf)
    mbs, mss = [], []
    for b in range(B):
        kk = (np.arange(n_kts[b])[None, :] * P + np.arange(P)[:, None]).astype(
            np.int64
        )
        vlb = int(vl[b])
        if vlb == 0:
            mbs.append(np.zeros((P, n_kts[b]), np.float32))
            mss.append(np.zeros((P, n_kts[b]), np.float32))
        else:
            mbs.append(np.where(kk < vlb, 0.0, NEG).astype(np.float32))
            mss.append(np.full((P, n_kts[b]), 1.0 / math.sqrt(HD), np.float32))
    mb = np.concatenate(mbs, axis=1)
    ms = np.concatenate(mss, axis=1)

    in_maps = []
    for c in range(NCORES):
        cols = slice(c * P, (c + 1) * P)
        in_maps.append(
            {
                "xq": xq,
                "xk": xk,
                "xv": xv,
                "wq": np.ascontiguousarray(Wq[:, cols]).astype(bf),
                "wk": np.ascontiguousarray(Wk[:, cols]).astype(bf),
                "wv": np.ascontiguousarray(Wv[:, cols]).astype(bf),
                "wo": np.ascontiguousarray(Wo[cols, :]).astype(bf),
                "mb": mb,
                "ms": ms,
            }
        )

    globals()["_LAST_IN_MAPS"] = in_maps
    res = run_bass_kernel_spmd(nc, in_maps, list(range(NCORES))).results

    out = np.zeros((B, S, D), dtype=np.float32)
    for c in range(NCORES):
        out += res[c]["out"].reshape(B, S, D).astype(np.float32)
    return out


# revision 43
# speedup vs baseline: 1.1870x; 1.1689x over previous
"""Multi-head attention (B=4, S=2048, D=1024, H=16) on 8 TRN2 NeuronCores.

Sharding: head-parallel. Core c owns heads {2c, 2c+1} (columns
[128c:128c+128] of Wq/Wk/Wv, rows of Wo) for ALL batches. Per-batch key
tile counts n_kt(b) = ceil(valid_len/128) are baked into the program, so
every core runs the same, minimal amount of attention work (sum over
batches of n_kt) instead of 8 heads x max(n_kt) as a batch-sharded
layout would. Row-parallel Wo gives per-core partial outputs [B,S,D]
summed on the host.

All matmul inputs are bf16 (inputs quantized host-side; rel err ~6e-3
vs the 2e-2 gate). Per-core dataflow:
  KT[d',s] = (Xk Wk)^T   (d' = 2*64 head dims on partitions)
  QT[d',s] = (Xq Wq)^T
  V[s,kt,h,65] = Xv Wv    (s on partitions; col 64 = ones -> denominator)
  per batch, per 512-wide q slab, per key tile kt:
    sc[k, 2, 512] = KT_h^T-slice.T @ QT_h  for both heads (two PE row groups)
    ex = Exp(sc*scale + maskbias)          (one ACT op covers both heads)
    av[0:65, h-half] += V_h.T @ ex_h       (row 64 accumulates denominator)
  normalization: denominators bounce through DRAM to land 32-wide on
  partitions for the DVE reciprocal, are broadcast via a K=1 matmul of
  ones, and multiply the head outputs into OT2[128, s] (head 1 hoisted
  to partitions 64:127 by an SBUF->SBUF DMA).
  out[s,:] partial = OT2[:, s-tile].T @ Wo2   (K=128, N=1024, bf16 out)

The PE instruction stream is kept gap-free (HAM clock at 8/8): the
projection matmuls of the next batch and the output-projection /
normalization of the previous batch are interleaved as filler between
attention rounds, so the PE never waits on the ACT engine's exp.
"""

import math

import numpy as np

B, S, D, H = 4, 2048, 1024, 16
HD = D // H  # 64
NCORES = 8
NEG = -1.0e6
P = 128

_PROG_CACHE = {}


def _patch_tile_drain():
    """The walrus build in this container rejects sem waits attached to the
    Tile end-of-kernel Drain ("Too many sync wait commands" / SIGABRT).
    Replace them with standalone EventSemaphore waits, which it accepts."""
    import concourse.tile as tile
    from concourse.vector_clock import ScopedClock

    if getattr(tile.TileContext, "_drain_patched", False):
        return

    def _drain_and_barrier(self, tick_clock, wait_clock):
        nc = self.nc
        drain_inst = nc.sync.drain()
        wait_clock.add_sem_waits(
            drain_inst.ins, ScopedClock({None: tick_clock.global_clock})
        )
        si = drain_inst.ins.sync_info
        waits = list(si.on_wait) if si is not None and si.on_wait else []
        if waits:
            si.on_wait.clear()
            by_id, by_name = {}, {}
            for h in wait_clock.sems.allocated().values():
                by_id[getattr(h, "id", None)] = h
                by_name[getattr(h, "name", None)] = h
            for w in waits:
                h = by_id.get(w.id) or by_name.get(w.ant_name)
                assert h is not None, f"no handle for sem {w.ant_name} ({w.id})"
                nc.sync.wait_ge(h, w.wait_value)
        nc.all_engine_barrier()
        assert self.sems is not None
        popped = nc._tile_sem_poison_stack.pop()
        assert popped is self._sem_poison
        nc.clear_and_free_semaphores(list(self.sems.allocated().values()))
        nc.all_engine_barrier()

    tile.TileContext._drain_and_barrier = _drain_and_barrier
    tile.TileContext._drain_patched = True


def _split_multi_waits(nc, mybir):
    """This container's walrus rejects instructions carrying more than one
    semaphore wait ("Too many sync wait commands"). Hoist excess waits into
    standalone EventSemaphore instructions on the same engine, inserted
    immediately before the instruction — same-engine stream order preserves
    the semantics exactly."""
    n_ev = 0
    for fn in nc.m.functions:
        for bb in fn.blocks:
            insts = bb.instructions
            out = []
            for inst in insts:
                si = inst.sync_info
                waits = list(si.on_wait) if si is not None and si.on_wait else []
                keep = 0 if inst.opcode == "Drain" else 1
                if len(waits) > keep:
                    excess = waits[: len(waits) - keep]
                    kept = waits[len(waits) - keep:]
                    si.on_wait.clear()
                    si.on_wait.extend(kept)
                    for w in excess:
                        ev = mybir.InstEventSemaphore(
                            name=f"{inst.name}-hw{n_ev}",
                            engine=inst.engine,
                        )
                        ev.sync_info = mybir.SyncInfo(on_wait=[w], on_update=[])
                        out.append(ev)
                        n_ev += 1
                out.append(inst)
            if n_ev:
                insts[:] = out
    return n_ev


def _build_program(n_kts: tuple):
    import concourse.bass as bass
    import concourse.mybir as mybir
    import concourse.tile as tile

    _patch_tile_drain()

    f32 = mybir.dt.float32
    f32r = mybir.dt.float32r
    bf16 = mybir.dt.bfloat16
    AF = mybir.ActivationFunctionType

    nks = sum(n_kts)
    koff = [sum(n_kts[:b]) for b in range(B)]  # packed kt-column offsets
    # biggest batch first: its wide attention window hides the next batch's
    # projection DMAs, and the accumulated tail work drains through the
    # remaining windows
    order = sorted(range(B), key=lambda b: -n_kts[b])

    nc = bass.Bass()

    xq_d = nc.dram_tensor("xq", [B, D, S], bf16, kind="ExternalInput")
    xk_d = nc.dram_tensor("xk", [D, nks * P], bf16, kind="ExternalInput")
    xv_d = nc.dram_tensor("xv", [D, nks * P], bf16, kind="ExternalInput")
    wq_d = nc.dram_tensor("wq", [D, P], bf16, kind="ExternalInput")
    wk_d = nc.dram_tensor("wk", [D, P], bf16, kind="ExternalInput")
    wv_d = nc.dram_tensor("wv", [D, P], bf16, kind="ExternalInput")
    wo_d = nc.dram_tensor("wo", [P, D], bf16, kind="ExternalInput")
    mb_d = nc.dram_tensor("mb", [P, nks], f32, kind="ExternalInput")
    ms_d = nc.dram_tensor("ms", [P, nks], f32, kind="ExternalInput")
    out_d = nc.dram_tensor("out", [B, 16, P, D], bf16, kind="ExternalOutput")

    with tile.TileContext(nc) as tc:
        with (
            tc.tile_pool(name="pp", bufs=1) as pp,
            tc.tile_pool(name="xsp", bufs=3) as xsp,
            tc.tile_pool(name="expp", bufs=3) as expp,
            tc.tile_pool(name="obp", bufs=3) as obp,
            tc.tile_pool(name="dnp", bufs=2) as dnp,
            tc.tile_pool(name="dnrp", bufs=3) as dnrp,
            tc.tile_pool(name="othp", bufs=2) as othp,
            tc.tile_pool(name="psMM", bufs=3, space="PSUM") as psMM,
            tc.tile_pool(name="psAV", bufs=1, space="PSUM") as psAV,
        ):
            # ---- persistent SBUF
            wq = pp.tile([P, 8, P], bf16, name="wq")
            wk = pp.tile([P, 8, P], bf16, name="wk")
            wv = pp.tile([P, 8, P], bf16, name="wv")
            wo = pp.tile([P, D], bf16, name="wo")
            ones = pp.tile([P, HD], bf16, name="ones")
            mb = pp.tile([P, nks], f32, name="mb")
            ms = pp.tile([P, nks], f32, name="ms")
            QT = [pp.tile([P, S], bf16, name=f"QT{b}") for b in range(B)]
            KT = [pp.tile([P, n_kts[b] * P], bf16, name=f"KT{b}") for b in range(B)]
            V = [pp.tile([P, n_kts[b], 2, HD + 1], bf16, name=f"V{b}") for b in range(B)]
            OT2 = [pp.tile([P, S], bf16, name=f"OT2{b}") for b in range(B)]
            avb_t = {}  # per-batch [65, 4, 2, 512] f32 tiles (pooled, bufs=2)
            dnr_t = {}  # per-batch [1, 2*S] f32 tiles (pooled, bufs=2)

            nc.any.memset(ones[:], 1.0)
            for b in range(B):
                nc.any.memset(V[b][:, :, :, HD:HD + 1], 1.0)
            nc.sync.dma_start(mb[:], mb_d[:, :])
            nc.sync.dma_start(ms[:], ms_d[:, :])
            nc.sync.dma_start(wq[:], wq_d[:, :].rearrange("(a p) c -> p a c", p=P))
            nc.sync.dma_start(wk[:], wk_d[:, :].rearrange("(a p) c -> p a c", p=P))
            nc.sync.dma_start(wv[:], wv_d[:, :].rearrange("(a p) c -> p a c", p=P))
            nc.sync.dma_start(wo[:], wo_d[:, :])

            xk_re = xk_d[:, :].rearrange("(a p) s -> p a s", p=P)
            xv_re = xv_d[:, :].rearrange("(a p) s -> p a s", p=P)

            ob_i = [0]  # alternates od copies between DVE and ACT

            # ---------- thunk builders (each thunk = one schedulable step) ----
            def a_thunks(b):
                """Projection phase for batch b: list of (dma_thunk, mm_thunk)
                pairs flattened with 2-ahead DMA prefetch."""
                dmas, mms = [], []
                xq_re = xq_d[b, :, :].rearrange("(a p) s -> p a s", p=P)
                nb = n_kts[b] * P

                # KT: (Xk Wk)^T in 512-col chunks
                for off in range(0, nb, 512):
                    cw = min(512, nb - off)
                    xs = [None]

                    def dk(off=off, cw=cw, xs=xs):
                        xs[0] = xsp.tile([P, 8, 512], bf16, name="xs", tag="xs")
                        nc.sync.dma_start(
                            xs[0][:, :, 0:cw],
                            xk_re[:, :, koff[b] * P + off:koff[b] * P + off + cw],
                        )

                    def mk(b=b, off=off, cw=cw, xs=xs):
                        pk = psMM.tile([P, 1024], f32, name="pk", tag="mm")
                        for a in range(8):
                            nc.tensor.matmul(
                                pk[:, 0:cw],
                                lhsT=wk[:, a, :],
                                rhs=xs[0][:, a, 0:cw],
                                start=(a == 0),
                                stop=(a == 7),
                            )
                        nc.vector.tensor_copy(
                            out=KT[b][:, off:off + cw], in_=pk[:, 0:cw]
                        )

                    dmas.append(dk)
                    mms.append(mk)

                # V: Xv Wv natural layout, 512-col chunks = 4 key tiles each
                for off in range(0, nb, 512):
                    cw = min(512, nb - off)
                    nst = cw // P
                    xs = [None]

                    def dv(off=off, cw=cw, xs=xs):
                        xs[0] = xsp.tile([P, 8, 512], bf16, name="xs", tag="xs")
                        nc.sync.dma_start(
                            xs[0][:, :, 0:cw],
                            xv_re[:, :, koff[b] * P + off:koff[b] * P + off + cw],
                        )

                    def mv(b=b, off=off, nst=nst, xs=xs):
                        pv = psMM.tile([P, 1024], f32, name="pv", tag="mm")
                        for st in range(nst):
                            for a in range(8):
                                nc.tensor.matmul(
                                    pv[:, st * P:(st + 1) * P],
                                    lhsT=xs[0][:, a, st * P:(st + 1) * P],
                                    rhs=wv[:, a, :],
                                    start=(a == 0),
                                    stop=(a == 7),
                                )
                        kt0 = off // P
                        nc.vector.tensor_copy(
                            out=V[b][:, kt0:kt0 + nst, :, 0:HD],
                            in_=pv[:, 0:nst * P].rearrange(
                                "p (st h c) -> p st h c", st=nst, h=2
                            ),
                        )

                    dmas.append(dv)
                    mms.append(mv)

                # QT: (Xq Wq)^T in 512-col chunks (4 chunks)
                for off in range(0, S, 512):
                    xs = [None]

                    def dq(b=b, off=off, xs=xs):
                        xs[0] = xsp.tile([P, 8, 512], bf16, name="xs", tag="xs")
                        nc.sync.dma_start(xs[0][:], xq_re[:, :, off:off + 512])

                    def mq(b=b, off=off, xs=xs):
                        pq = psMM.tile([P, 1024], f32, name="pq", tag="mm")
                        for a in range(8):
                            nc.tensor.matmul(
                                pq[:, 0:512],
                                lhsT=wq[:, a, :],
                                rhs=xs[0][:, a, :],
                                start=(a == 0),
                                stop=(a == 7),
                            )
                        nc.vector.tensor_copy(
                            out=QT[b][:, off:off + 512], in_=pq[:, 0:512]
                        )

                    dmas.append(dq)
                    mms.append(mq)

                # interleave: keep 2 DMAs in flight ahead of their consumers
                ahead = 2
                thunks = []
                nd = len(dmas)
                for i in range(min(ahead, nd)):
                    thunks.append(dmas[i])
                for i in range(nd):
                    thunks.append(mms[i])
                    if i + ahead < nd:
                        thunks.append(dmas[i + ahead])
                return thunks

            oth_t = {}  # per-batch [HD, S] bf16 head-1 staging (pooled)

            def slab_tail_thunks(b, slab):
                """Normalize + output-project one finished 512-q slab of batch
                b. Returns PE-bearing thunks for the filler queue; the
                denominator reciprocal chain (DMA/DVE only) is emitted inline
                by the caller so its latency hides behind later rounds."""
                q0 = slab * 512
                ts = []

                def tm(b=b, slab=slab, q0=q0):
                    bc = psMM.tile([P, 1024], f32, name="bc", tag="mm")
                    if slab == 0:
                        oth_t[b] = othp.tile([HD, S], bf16, name="oth", tag="oth")
                    for h in range(2):
                        nc.tensor.matmul(
                            bc[0:HD, h * 512:(h + 1) * 512],
                            lhsT=ones[0:1, :],
                            rhs=dnr_t[(b, slab)][0:1, h, :],
                            start=True,
                            stop=True,
                        )
                    for h in range(2):
                        dst = OT2[b] if h == 0 else oth_t[b]
                        nc.vector.tensor_mul(
                            out=dst[0:HD, q0:q0 + 512],
                            in0=avb_t[b][0:HD, slab, h, :],
                            in1=bc[0:HD, h * 512:(h + 1) * 512],
                        )
                    nc.sync.dma_start(
                        OT2[b][HD:P, q0:q0 + 512], oth_t[b][:, q0:q0 + 512]
                    )

                ts.append(tm)
                for st in range(4 * slab, 4 * slab + 4):

                    def td(b=b, st=st):
                        od = psMM.tile([P, 1024], f32, name="od", tag="mm")
                        for u in range(2):  # ISA moving-operand max is 512
                            nc.tensor.matmul(
                                od[:, u * 512:(u + 1) * 512],
                                lhsT=OT2[b][:, st * P:(st + 1) * P],
                                rhs=wo[:, u * 512:(u + 1) * 512],
                                start=True,
                                stop=True,
                            )
                        ob = obp.tile([P, 1024], bf16, name="ob", tag="ob")
                        if ob_i[0] % 2 == 0:
                            nc.vector.tensor_copy(out=ob[:], in_=od[:])
                        else:
                            nc.scalar.copy(ob[:], od[:])
                        ob_i[0] += 1
                        nc.sync.dma_start(out_d[b, st, :, :], ob[:])

                    ts.append(td)
                return ts

            # ---------- attention phase with filler interleaving ------------
            gr = [0]  # global round counter (paces filler readiness)

            def run_c(b, fill):
                """Attention for batch b; pops filler thunks between rounds so
                the PE stream never drains while ACT computes exp. Entries are
                (ready_round, thunk): a thunk only pops once the global round
                counter passes ready_round, keeping chain latencies hidden."""
                nkt = n_kts[b]
                rounds = 4 * nkt
                r = 0
                for slab in range(4):
                    q0 = slab * 512
                    avs = psAV.tile([HD + 1, 1024], f32, name="avs", tag="av")
                    pend = None  # (kt, ex) awaiting its AV matmuls
                    for kt in range(nkt):
                        sc = psMM.tile([P, 1024], f32, name="sc", tag="mm")
                        for h in range(2):
                            nc.tensor.matmul(
                                sc[:, h * 512:(h + 1) * 512],
                                lhsT=KT[b][h * HD:(h + 1) * HD, kt * P:(kt + 1) * P],
                                rhs=QT[b][h * HD:(h + 1) * HD, q0:q0 + 512],
                                start=True,
                                stop=True,
                            )
                        ex = expp.tile([P, 1024], bf16, name="ex", tag="ex")
                        col = koff[b] + kt
                        nc.scalar.activation(
                            ex[:],
                            sc[:],
                            AF.Exp,
                            bias=mb[:, col:col + 1],
                            scale=ms[:, col:col + 1],
                        )
                        if pend is not None:
                            pkt, pex = pend
                            for h in range(2):
                                nc.tensor.matmul(
                                    avs[:, h * 512:(h + 1) * 512],
                                    lhsT=V[b][:, pkt, h, :],
                                    rhs=pex[:, h * 512:(h + 1) * 512],
                                    start=(pkt == 0),
                                    stop=False,
                                )
                        pend = (kt, ex)
                        # filler keeps the PE busy while ACT runs exp; pop the
                        # first READY entry (skip entries still inside their
                        # producer chain's latency window — same-group entries
                        # share a ready round, so group order is preserved)
                        rem_rounds = rounds - r
                        quota = (len(fill) + rem_rounds - 1) // rem_rounds
                        done = 0
                        while fill and done < quota:
                            idx = next(
                                (k for k, e in enumerate(fill) if e[0] <= gr[0]),
                                None,
                            )
                            if idx is None:
                                break
                            fill.pop(idx)[1]()
                            done += 1
                        r += 1
                        gr[0] += 1
                    pkt, pex = pend
                    for h in range(2):
                        nc.tensor.matmul(
                            avs[:, h * 512:(h + 1) * 512],
                            lhsT=V[b][:, pkt, h, :],
                            rhs=pex[:, h * 512:(h + 1) * 512],
                            start=(pkt == 0),
                            stop=True,
                        )
                    # release the PSUM tile: head outputs and denominator row
                    if slab == 0:
                        avb_t[b] = dnp.tile(
                            [HD + 1, 4, 2, 512], f32, name="avb", tag="avb"
                        )
                    nc.vector.tensor_copy(
                        out=avb_t[b][:, slab, :, :],
                        in_=avs[:, :].rearrange("p (h q) -> p h q", h=2),
                    )
                    # denominator reciprocal: bounce the row across 8
                    # partitions (SBUF->SBUF DMA) so the 6-cycle/elem DVE
                    # reciprocal runs 8-wide, then collapse back. The
                    # dependent broadcast matmul is delayed ~4 rounds in the
                    # filler queue so this chain's latency stays hidden.
                    dnT = dnrp.tile([8, P], f32, name="dnT", tag="dnT")
                    dnTr = dnrp.tile([8, P], bf16, name="dnTr", tag="dnTr")
                    nc.sync.dma_start(dnT[:], avb_t[b][HD:HD + 1, slab, :, :])
                    with nc.allow_low_precision(
                        reason="softmax denominators are O(1e3); bf16 "
                        "reciprocal keeps enough digits for attention"
                    ):
                        nc.vector.reciprocal(dnTr[:], dnT[:])
                    dnr = dnrp.tile([1, 2, 512], bf16, name="dnr", tag="dnr")
                    nc.sync.dma_start(dnr[:], dnTr[:])
                    dnr_t[(b, slab)] = dnr
                    fill.extend(
                        (gr[0] + 4, t) for t in slab_tail_thunks(b, slab)
                    )

            # ---------- schedule ---------------------------------------------
            fill = []
            for t in a_thunks(order[0]):
                t()
            for i, b in enumerate(order):
                if i + 1 < B:
                    fill.extend((0, t) for t in a_thunks(order[i + 1]))
                run_c(b, fill)
            while fill:
                fill.pop(0)[1]()

    _split_multi_waits(nc, mybir)
    return nc


def _zip(l1, l2):
    out = []
    for i in range(max(len(l1), len(l2))):
        if i < len(l1):
            out.append(l1[i])
        if i < len(l2):
            out.append(l2[i])
    return out


def _get_program(n_kts: tuple):
    if n_kts not in _PROG_CACHE:
        _PROG_CACHE[n_kts] = _build_program(n_kts)
    return _PROG_CACHE[n_kts]


def kernel(**inputs) -> np.ndarray:
    import ml_dtypes
    from concourse.bass_utils import run_bass_kernel_spmd

    bf = ml_dtypes.bfloat16
    q = np.asarray(inputs["queries"], dtype=np.float32)
    k = np.asarray(inputs["keys"], dtype=np.float32)
    v = np.asarray(inputs["values"], dtype=np.float32)
    vl = np.asarray(inputs["valid_lens"]).astype(np.int64)
    Wq = np.asarray(inputs["Wq"], dtype=np.float32)
    Wk = np.asarray(inputs["Wk"], dtype=np.float32)
    Wv = np.asarray(inputs["Wv"], dtype=np.float32)
    Wo = np.asarray(inputs["Wo"], dtype=np.float32)

    n_kts = tuple(
        (S // P) if int(vl[b]) == 0 else min(S // P, math.ceil(int(vl[b]) / P))
        for b in range(B)
    )
    nc = _get_program(n_kts)
    nks = sum(n_kts)

    # shared (core-independent) input tensors
    xq = np.ascontiguousarray(q.transpose(0, 2, 1)).astype(bf)
    xk = np.concatenate(
        [k[b].T[:, :n_kts[b] * P] for b in range(B)], axis=1
    ).astype(bf)
    xv = np.concatenate(
        [v[b].T[:, :n_kts[b] * P] for b in range(B)], axis=1
    ).astype(bf)
    mbs, mss = [], []
    for b in range(B):
        kk = (np.arange(n_kts[b])[None, :] * P + np.arange(P)[:, None]).astype(
            np.int64
        )
        vlb = int(vl[b])
        if vlb == 0:
            mbs.append(np.zeros((P, n_kts[b]), np.float32))
            mss.append(np.zeros((P, n_kts[b]), np.float32))
        else:
            mbs.append(np.where(kk < vlb, 0.0, NEG).astype(np.float32))
            mss.append(np.full((P, n_kts[b]), 1.0 / math.sqrt(HD), np.float32))
    mb = np.concatenate(mbs, axis=1)
    ms = np.concatenate(mss, axis=1)

    in_maps = []
    for c in range(NCORES):
        cols = slice(c * P, (c + 1) * P)
        in_maps.append(
            {
                "xq": xq,
                "xk": xk,
                "xv": xv,
                "wq": np.ascontiguousarray(Wq[:, cols]).astype(bf),
                "wk": np.ascontiguousarray(Wk[:, cols]).astype(bf),
                "wv": np.ascontiguousarray(Wv[:, cols]).astype(bf),
                "wo": np.ascontiguousarray(Wo[cols, :]).astype(bf),
                "mb": mb,
                "ms": ms,
            }
        )

    globals()["_LAST_IN_MAPS"] = in_maps
    res = run_bass_kernel_spmd(nc, in_maps, list(range(NCORES))).results

    out = np.zeros((B, S, D), dtype=np.float32)
    for c in range(NCORES):
        out += res[c]["out"].reshape(B, S, D).astype(np.float32)
    return out
